# revision 20
# baseline (speedup 1.0000x reference)
"""MeshPotential (P3M) Trainium2 kernel, v6: atom-direct truncated-mode
pipeline with budgeted disk truncation.

G(k) = 4*pi*exp(-sigma^2 k^2/2)/k^2 decays as exp(-0.0079 n^2) on this
mesh, so only low modes survive.  The whole computation runs in mode
space per atom -- no 256^3 mesh, no binning:

  rho_hat(k)  = sum_n q_n Sx[n,kx] Sy[n,ky] Sz[n,kz]     (spread)
  V(k)        = G(k) * rho_hat(k)                         (convolution)
  out_n       = sum_k Re( conj(S_n(k)) V(k) )             (gather)

with S the separable order-4 stencil DFT factors, computed on host.
kx runs -K..K (KK wide); the (ky,kz) plane (kz>=0, Hermitian weight 2)
is truncated to the UTOT lowest-|ky,kz| rows and dealt evenly across
the two kz-half cores, 256 rows (2 psum chunks) each.

Device stages per core (core = (channel, row-half), 8 cores, SPMD):
  P1 spread   rhoT[u,kx] = sum_n T[n,u] SxW[n,kx]
  P2 G-mult   VTA[u] = [G*rho_r | -G*rho_i]      (DVE psum cast)
              VTB[u] = [G*rho_i |  G*rho_r]      (DVE + Act copy)
  P3 gather   wt[n, {r,i'}] += T6r[u,n]^T [Vr|Vni] + T6i[u,n]^T [Vi|Vr]
  P4 dot      out_n = sum_kx {SxR6,-SxI6} * wt   (DVE mult+reduce)

All matmuls use full 128-row contraction and 128-wide stationary
operands; 5 consolidated input DMAs over 3 queues; gather runs in two
4-bank waves interleaved per u-chunk so it overlaps the t6 stream.
"""

import numpy as np
import ml_dtypes

import concourse.bass as bass  # noqa: F401
import concourse.mybir as mybir
import concourse.tile as tile
from concourse import bacc, masks
from concourse.bass_utils import run_bass_kernel_spmd

F32 = mybir.dt.float32
BF16 = mybir.dt.bfloat16
NPBF = ml_dtypes.bfloat16

NS = 256
SMEARING = 0.4
N_CORES = 8

K = 20
KK = 2 * K + 1            # 41 kx modes -K..K
KZ = K + 1                # kz 0..K (Hermitian weight 2 for kz>0)
UTOT = 512                # kept (ky,kz) rows, lowest ky^2+kz^2
U4 = UTOT // 2            # 256 rows per core
NUC = U4 // 128           # 2 u-chunks
NGC = 8                   # gather chunks of 128 atoms (1024 padded)
SPW = 2 * U4 + 3 * KK     # spr row: [Tr | Ti | SxR | SxI | -SxI]

_cache = {}


def build_program(NSPC):
    nc = bacc.Bacc(None, target_bir_lowering=False, debug=False)
    dp = lambda name, shape, dt=BF16: nc.declare_dram_parameter(
        name, list(shape), dt, isOutput=False)
    spr = dp("spr", (128, NSPC, SPW))           # spread lhsT+rhs, one DMA
    g3 = dp("g3", (128, NUC, 3, KK))            # {G, -G, G} by u row
    t6 = dp("t6", (128, NUC, 2, 1024))          # gather lhsT: {T6r,T6i} x atom
    sx6 = dp("sx6", (128, NGC, 2, KK))          # dot in1: {SxR6, -SxI6}
    outp = nc.declare_dram_parameter("out", [NGC, 128], F32, isOutput=True)
    mult = mybir.AluOpType.mult
    add = mybir.AluOpType.add

    with tile.TileContext(nc) as tc:
        with (
            tc.tile_pool(name="constp", bufs=1) as constp,
            tc.tile_pool(name="iop", bufs=2) as iop,
            tc.tile_pool(name="psp", bufs=1, space="PSUM") as psp,
        ):
            SPR = constp.tile([128, NSPC, SPW], BF16)
            G3 = constp.tile([128, NUC, 3, KK], BF16)
            T6 = constp.tile([128, NUC, 2, 1024], BF16)
            SX6 = constp.tile([128, NGC, 2, KK], BF16)
            OUT = constp.tile([128, NGC], F32)
            OUTT = constp.tile([NGC, 128], F32)
            VTA = constp.tile([128, NUC, 2, KK], BF16)
            VTB = constp.tile([128, NUC, 2, KK], BF16)
            IDN = constp.tile([128, 128], F32)
            WUP = constp.tile([128, 128], BF16)

            # consolidated input DMAs; spr ahead of t6 on the same queue
            # so its transfer takes the full DMA-engine fanout first
            nc.scalar.dma_start(SPR[:], spr[:])
            nc.scalar.dma_start(T6[:, 0], t6[:, 0])
            nc.scalar.dma_start(T6[:, 1], t6[:, 1])
            nc.gpsimd.dma_start(G3[:], g3[:])
            nc.gpsimd.dma_start(SX6[:], sx6[:])

            mm = nc.tensor.matmul

            # PE warm-up: dummy matmuls on zeros while the DMAs land, so
            # DVFS is ramped when the real pipeline starts
            nc.gpsimd.memset(WUP[:], 0.0)
            masks.make_identity(nc, IDN[:])
            ps_w = psp.tile([128, 128], F32, tag="D")
            for _ in range(16):
                mm(ps_w[:], WUP[:], WUP[:], start=True, stop=True)

            # ---- P1 spread + P2 G-mult, per u-chunk --------------------
            ps_s = psp.tile([128, NUC, 2, KK], F32, tag="S")
            for ci in range(NUC):
                # one accumulation group open per psum bank at a time
                for ri, (b0, b1) in enumerate(((0, 2), (1, 0))):
                    for j in range(NSPC):
                        lh = lambda zri: SPR[:, j, zri * U4 + ci * 128:
                                             zri * U4 + (ci + 1) * 128]
                        rh = lambda b: SPR[:, j, 2 * U4 + b * KK:
                                           2 * U4 + (b + 1) * KK]
                        mm(ps_s[:, ci, ri], lh(0), rh(b0),
                           start=j == 0, stop=False)
                        mm(ps_s[:, ci, ri], lh(1), rh(b1),
                           start=False, stop=j == NSPC - 1)
                # VTA = [G*rho_r | -G*rho_i]   (one DVE op, psum -> bf16)
                nc.vector.tensor_tensor(VTA[:, ci], ps_s[:, ci],
                                        G3[:, ci, 0:2], op=mult)
                # VTB = [G*rho_i | G*rho_r]
                nc.vector.tensor_tensor(VTB[:, ci, 0:1], ps_s[:, ci, 1:2],
                                        G3[:, ci, 2:3], op=mult)
                nc.vector.tensor_tensor(VTB[:, ci, 1:2], ps_s[:, ci, 0:1],
                                        G3[:, ci, 2:3], op=mult)

            # ---- P3 gather + P4 dot: two 4-bank waves, u-chunk outer ---
            for wv in range(2):
                wts = [psp.tile([128, 2, KK], F32, tag=f"W{gj}",
                                name=f"wt{wv}{gj}")
                       for gj in range(4)]
                for ci in range(NUC):
                    for gj in range(4):
                        gi = wv * 4 + gj
                        gsl = slice(gi * 128, (gi + 1) * 128)
                        mm(wts[gj][:], T6[:, ci, 0, gsl], VTA[:, ci],
                           start=ci == 0, stop=False)
                        mm(wts[gj][:], T6[:, ci, 1, gsl], VTB[:, ci],
                           start=False, stop=ci == NUC - 1)
                for gj in range(4):
                    gi = wv * 4 + gj
                    scr = iop.tile([128, 2, KK], BF16, tag=f"s{gj % 2}")
                    nc.vector.tensor_tensor(scr[:], wts[gj][:], SX6[:, gi],
                                            op=mult)
                    nc.vector.tensor_reduce(OUT[:, gi:gi + 1], scr[:],
                                            axis=mybir.AxisListType.XY, op=add)
            # transpose the result so the output DMA is NGC fat rows
            # instead of 128 tiny per-partition descriptors
            ps_t = psp.tile([NGC, 128], F32, tag="T", name="ps_t")
            nc.tensor.transpose(ps_t[:], OUT[:], IDN[:])
            nc.vector.tensor_copy(OUTT[:], ps_t[:])
            nc.sync.dma_start(outp[:], OUTT[:])
    nc.compile()
    return nc


def _weights_order4(x):
    x2 = x * x
    x3 = x2 * x
    return np.stack([
        (1 - 6 * x + 12 * x2 - 8 * x3) / 48,
        (23 - 30 * x - 12 * x2 + 24 * x3) / 48,
        (23 + 30 * x - 12 * x2 - 24 * x3) / 48,
        (1 + 6 * x + 12 * x2 + 8 * x3) / 48,
    ])


def host_prep(cell, positions, charges):
    NA = positions.shape[0]
    NSP = charges.shape[1]
    cell = np.asarray(cell, dtype=np.float64)
    positions = np.asarray(positions, dtype=np.float64)
    charges = np.asarray(charges, dtype=np.float64)

    inv_cell = np.linalg.inv(cell)
    pos_rel = NS * (positions @ inv_cell)
    idx0 = np.floor(pos_rel)
    t = pos_rel - (idx0 + 0.5)
    w = _weights_order4(t)                                   # (4, NA, 3)
    offs = np.arange(-1, 3)
    idx = (idx0.astype(np.int64)[None] + offs[:, None, None]) % NS

    mo = np.arange(-K, K + 1)
    moz = np.arange(0, K + 1)

    def dft(ax, modes):
        ph = np.exp(-2j * np.pi * idx[:, :, ax][..., None] * modes / NS)
        return np.einsum("jn,jnm->nm", w[:, :, ax], ph)      # (NA, M)

    Sx = dft(0, mo)
    Sy = dft(1, mo)
    Sz = dft(2, moz)

    # smeared Coulomb kernel; Hermitian-z weight and 1/det folded in
    recip = 2 * np.pi * inv_cell.T
    kxg, kyg, kzg = np.meshgrid(mo.astype(np.float64), mo.astype(np.float64),
                                moz.astype(np.float64), indexing="ij")
    kvec = (kxg[..., None] * recip[0] + kyg[..., None] * recip[1]
            + kzg[..., None] * recip[2])
    ksq = np.sum(kvec * kvec, axis=-1)
    G = np.where(ksq == 0, 0.0,
                 4 * np.pi * np.exp(-0.5 * SMEARING ** 2 * ksq)
                 / np.where(ksq == 0, 1.0, ksq))
    G = G / np.abs(np.linalg.det(cell))
    wkz = np.where(moz == 0, 1.0, 2.0)
    Gw = G * wkz                                             # (KK, KK, KZ)

    # keep the UTOT lowest-|ky,kz| rows, dealt alternately to the halves
    r2 = (mo[:, None] ** 2 + moz[None, :] ** 2).ravel()
    order = np.argsort(r2, kind="stable")[:UTOT]
    halves = [order[0::2], order[1::2]]                      # U4 rows each
    yix = [h // KZ for h in halves]
    zix = [h % KZ for h in halves]

    sx6 = np.zeros((1024, 2, KK), dtype=NPBF)
    sx6[:NA, 0] = Sx.real
    sx6[:NA, 1] = -Sx.imag
    sx6 = np.ascontiguousarray(
        sx6.reshape(NGC, 128, 2, KK).transpose(1, 0, 2, 3))  # (128,NGC,2,KK)

    per_h = []
    for h in range(2):
        TYZ = Sy[:, yix[h]] * Sz[:, zix[h]]                  # (NA, U4)
        t6 = np.zeros((U4, 2, 1024), dtype=NPBF)
        t6[:, 0, :NA] = TYZ.real.T
        t6[:, 1, :NA] = TYZ.imag.T
        t6 = np.ascontiguousarray(
            t6.reshape(NUC, 128, 2, 1024).transpose(1, 0, 2, 3))
        gs = Gw[:, yix[h], zix[h]].T                         # (U4, KK)
        g3 = np.stack([gs, -gs, gs], axis=1).astype(NPBF)    # (U4, 3, KK)
        g3 = np.ascontiguousarray(
            g3.reshape(NUC, 128, 3, KK).transpose(1, 0, 2, 3))
        per_h.append((TYZ, t6, g3))

    NSPC = 1
    sels = []
    for c in range(NSP):
        sel = np.where(charges[:, c] != 0)[0]
        sels.append(sel)
        NSPC = max(NSPC, (len(sel) + 127) // 128)

    per_core = []
    for c in range(NSP):
        sel = sels[c]
        q = charges[sel, c]
        SxW = Sx[sel] * q[:, None]
        sxw = np.zeros((NSPC * 128, 3, KK))
        sxw[:len(sel), 0] = SxW.real
        sxw[:len(sel), 1] = SxW.imag
        sxw[:len(sel), 2] = -SxW.imag
        for h in range(2):
            TYZ = per_h[h][0]
            spra = np.zeros((NSPC * 128, SPW), dtype=NPBF)
            spra[:len(sel), 0:U4] = TYZ.real[sel]
            spra[:len(sel), U4:2 * U4] = TYZ.imag[sel]
            spra[:, 2 * U4:] = sxw.reshape(NSPC * 128, 3 * KK)
            spra = np.ascontiguousarray(
                spra.reshape(NSPC, 128, SPW).transpose(1, 0, 2))
            per_core.append({
                "spr": spra, "g3": per_h[h][2], "t6": per_h[h][1], "sx6": sx6,
            })
    return NSPC, NA, NSP, per_core


def _run(cell, positions, charges, trace=False):
    NSPC, NA, NSP, in_maps = host_prep(cell, positions, charges)
    if NSPC not in _cache:
        _cache[NSPC] = build_program(NSPC)
    nc = _cache[NSPC]

    res = run_bass_kernel_spmd(nc, in_maps, list(range(N_CORES)), trace=trace)

    pot = np.zeros((NA, NSP), dtype=np.float64)
    for core in range(N_CORES):
        c = core // 2
        out = res.results[core]["out"].astype(np.float64)    # (NGC, 128)
        pot[:, c] += out.reshape(-1)[:NA]
    return pot.astype(np.float32), res


def kernel(cell, positions, charges):
    pot, _ = _run(cell, positions, charges, trace=False)
    return pot
